# revision 18
# baseline (speedup 1.0000x reference)
"""ALSH net forward on 8 Trainium2 NeuronCores (bass, SPMD data-parallel).

Strategy
--------
Data-parallel over the batch.  Samples are grouped by their layer-1 hash
bucket on the host (hashes computed with the exact same jax ops as the
reference so the bucket assignments are bit-identical), then dealt
round-robin to the 8 cores so every core sees the same per-bucket counts
(padded) and a single SPMD graph serves all cores.

Per core the kernel exploits the ALSH block-sparsity: a sample only needs
the weight rows in its own bucket, so layer 1 runs ~10x fewer FLOPs than
the dense matmul.  Numerical reproducibility (hash buckets flip on 1-ulp
differences) is handled by replicating the neuron-XLA fp32 matmul
structure exactly: x-tiles stationary, K chunked by 128 in ascending
order, PSUM-accumulated; the layer-2 query hash is computed from a
densely scattered h1^T (built with bit-preserving 0/1 scatter matmuls)
with the same chunk structure the reference's matvec uses.
"""

import numpy as np

R = 0.1
TABLE = 10
M = 5
NCORES = 8
B, D, H, O = 16384, 3072, 1000, 10
KCH = D // 128  # 24 K chunks for layer 1


def _host_hashes(x, W1, W2, a1, bh1, a2, bh2):
    """Bit-exact replica of the reference hash computations (same jnp ops,
    same neuron backend) for the quantities that only depend on inputs."""
    import jax.numpy as jnp

    def _P(w):
        n = jnp.linalg.norm(w, axis=-1, keepdims=True)
        powers = jnp.concatenate([n ** (2 ** (i + 1)) for i in range(M)], axis=-1)
        return jnp.concatenate([w, powers], axis=-1)

    def _Q(xx):
        halves = jnp.full(xx.shape[:-1] + (M,), 0.5, xx.dtype)
        return jnp.concatenate([xx, halves], axis=-1)

    def _hash(v, a, b):
        h = jnp.floor((v @ a + b) / R)
        return h.astype(jnp.int32) % TABLE

    row_h1 = np.asarray(_hash(_P(jnp.asarray(W1)), jnp.asarray(a1), jnp.asarray(bh1)))
    row_h2 = np.asarray(_hash(_P(jnp.asarray(W2)), jnp.asarray(a2), jnp.asarray(bh2)))
    q_h1 = np.asarray(_hash(_Q(jnp.asarray(x)), jnp.asarray(a1), jnp.asarray(bh1)))
    return row_h1, row_h2, q_h1


def _build_graph(plan):
    import sys
    if '/opt/trn_rl_repo' not in sys.path:
        sys.path.insert(0, '/opt/trn_rl_repo')
    import concourse.bass as bass
    import concourse.mybir as mybir
    from contextlib import ExitStack

    F32 = mybir.dt.float32
    I32 = mybir.dt.int32
    AF = mybir.ActivationFunctionType
    ALU = mybir.AluOpType

    Bp = plan['Bp']
    nW = plan['nW']            # [10] rows per bucket (layer1, perm1 order)
    rc0 = plan['rc0']          # [10] start row of bucket in perm1 order
    coloff = plan['coloff']    # [10] per-core sample column offset per bucket
    mstar = plan['mstar']      # [10] per-core padded sample count per bucket
    tiles = plan['tiles']      # [(c, m0, mlen)] layer-1 sample tiles (20)
    l1bank = plan['l1bank']    # per tile: (bank, freeoff)
    ccs = plan['ccs']          # [(off, len)] q2dot column chunks over Bp
    bh2 = plan['bh2']
    W2D = plan['W2D']          # hash-chain 2d width = ceil(Bp/128)
    NX, NWS = 3, 2             # x / w1t dma splits per chunk
    a2len = [128] * 7 + [109]

    nc = bass.Bass()
    xt_ext = nc.dram_tensor("xt", [D, Bp], F32, kind="ExternalInput")
    w1_ext = nc.dram_tensor("w1t", [D, H], F32, kind="ExternalInput")
    w2_ext = nc.dram_tensor("w2t", [H, 1024], F32, kind="ExternalInput")
    wo_ext = nc.dram_tensor("wot", [H, O], F32, kind="ExternalInput")
    sm_ext = [nc.dram_tensor(f"smat{c}", [int(nW[c]), H], F32, kind="ExternalInput")
              for c in range(10)]
    a2_ext = nc.dram_tensor("a2c", [1005, 1], F32, kind="ExternalInput")
    rb_ext = nc.dram_tensor("rowb", [128, 8], F32, kind="ExternalInput")
    hv_ext = nc.dram_tensor("halves", [5, Bp], F32, kind="ExternalInput")
    on_ext = nc.dram_tensor("ones", [1, 128], F32, kind="ExternalInput")
    id_ext = nc.dram_tensor("ident", [128, 128], F32, kind="ExternalInput")
    out_ext = nc.dram_tensor("out", [O, Bp], F32, kind="ExternalOutput")
    hscr_ext = nc.dram_tensor("hscr", [128 * W2D], F32)
    pscr_ext = nc.dram_tensor("pscr", [128 * W2D], F32)

    ctx = ExitStack()
    ec = ctx.enter_context
    xtb = ec(nc.sbuf_tensor("xtb", [128, 2 * KCH * 128], F32))
    wcb = ec(nc.sbuf_tensor("wcb", [128, 2 * KCH * 128], F32))
    idsb = ec(nc.sbuf_tensor("idsb", [128, 128], F32))
    onsb = ec(nc.sbuf_tensor("onsb", [1, 128], F32))
    rbsb = ec(nc.sbuf_tensor("rbsb", [128, 8], F32))
    wosb = ec(nc.sbuf_tensor("wosb", [128, 8 * O], F32))
    w2sb = ec(nc.sbuf_tensor("w2sb", [128, 2 * 1024], F32))
    smsb = ec(nc.sbuf_tensor("smsb", [128, 10 * H], F32))
    a2sb = ec(nc.sbuf_tensor("a2sb", [128, 8], F32))
    yblk = ec(nc.sbuf_tensor("yblk", [128, sum(int(nW[c]) for c, _, _ in tiles)], F32))
    hcomp = ec(nc.sbuf_tensor("hcomp", [128, Bp], F32))
    dense = [ec(nc.sbuf_tensor(f"dense{k}", [128, Bp], F32)) for k in range(2)]
    q2row = ec(nc.sbuf_tensor("q2row", [1, 128 * W2D], F32))
    pmrow = ec(nc.sbuf_tensor("pmrow", [1, 128 * W2D], F32))
    hs = ec(nc.sbuf_tensor("hs", [128, 12 * W2D], F32))
    hsi = ec(nc.sbuf_tensor("hsi", [128, 2 * W2D], I32))
    pmb = ec(nc.sbuf_tensor("pmb", [128, Bp], F32))
    eqs = ec(nc.sbuf_tensor("eqs", [128, 2 * 256], F32))
    h2s = ec(nc.sbuf_tensor("h2s", [128, 2 * 256], F32))
    h2t = ec(nc.sbuf_tensor("h2t", [128, 2 * 8 * 256], F32))
    outsb = ec(nc.sbuf_tensor("outsb", [10, Bp], F32))
    banks = [ec(nc.psum_tensor(f"bank{i}", [128, 512], F32)) for i in range(8)]

    dsem = ec(nc.semaphore("dsem"))
    hd = ec(nc.semaphore("hd"))
    pk = ec(nc.semaphore("pk"))
    ar = ec(nc.semaphore("ar"))
    pt = ec(nc.semaphore("pt"))
    vc = ec(nc.semaphore("vc"))
    psc = ec(nc.semaphore("psc"))
    vd = ec(nc.semaphore("vd"))
    pq = ec(nc.semaphore("pq"))
    vq = ec(nc.semaphore("vq"))
    hz = ec(nc.semaphore("hz"))
    ah = ec(nc.semaphore("ah"))
    pb = ec(nc.semaphore("pb"))
    vb = ec(nc.semaphore("vb"))
    pe2 = ec(nc.semaphore("pe2"))
    hz2 = ec(nc.semaphore("hz2"))
    vm = ec(nc.semaphore("vm"))
    am = ec(nc.semaphore("am"))
    pe3 = ec(nc.semaphore("pe3"))
    vo = ec(nc.semaphore("vo"))
    hv = ec(nc.semaphore("hv"))
    dxa = ec(nc.semaphore("dxa"))
    dxb = ec(nc.semaphore("dxb"))
    dwa = ec(nc.semaphore("dwa"))
    dwb = ec(nc.semaphore("dwb"))
    dw2 = ec(nc.semaphore("dw2"))
    dout = ec(nc.semaphore("dout"))
    block = ec(nc.Block())

    # yblk free offsets per tile
    ybo = []
    o_ = 0
    for (c, m0, mlen) in tiles:
        ybo.append(o_)
        o_ += int(nW[c])

    # layer-2 halves of each bucket's sample columns
    halves2 = []
    for c in range(10):
        m = int(mstar[c])
        h0 = (m + 1) // 2
        halves2.append([(0, h0), (h0, m - h0)])

    D0 = 29 * 16  # phase-0 dma completions

    # ---------------- sync engine: all input DMAs ----------------
    @block.sync
    def _(s):
        s.dma_start(idsb[:, :], id_ext[:, :]).then_inc(dsem, 16)
        s.dma_start(onsb[:, :], on_ext[:, :]).then_inc(dsem, 16)
        s.dma_start(rbsb[:, :], rb_ext[:, :]).then_inc(dsem, 16)
        for t in range(8):
            klen = min(128, H - t * 128)
            s.dma_start(wosb[0:klen, t * O:(t + 1) * O],
                        wo_ext[t * 128:t * 128 + klen, :]).then_inc(dsem, 16)
        for c in range(10):
            s.dma_start(smsb[0:int(nW[c]), c * H:(c + 1) * H],
                        sm_ext[c][:, :]).then_inc(dsem, 16)
        for k in range(8):
            ck = a2len[k]
            s.dma_start(a2sb[0:ck, k:k + 1],
                        a2_ext[k * 128:k * 128 + ck, :]).then_inc(dsem, 16)
        # streamed layer-1 tiles: x columns per tile, W columns per bucket
        xt_v = xt_ext.rearrange("(k p) m -> p k m", p=128)
        w1_v = w1_ext.rearrange("(k p) m -> p k m", p=128)
        first_tile = {}
        last_tile = {}
        for t_, (c_, _, _) in enumerate(tiles):
            first_tile.setdefault(c_, t_)
            last_tile[c_] = t_
        for t_, (c_, m0_, ml_) in enumerate(tiles):
            if first_tile[c_] == t_:
                dw = dwa if c_ % 2 == 0 else dwb
                if c_ >= 2:
                    s.wait_ge(pk, last_tile[c_ - 2] + 1)
                for i in range(NWS):
                    k0, k1 = (i * KCH) // NWS, ((i + 1) * KCH) // NWS
                    s.dma_start(
                        wcb[:, (c_ % 2) * KCH * 128 + k0 * 128:
                            (c_ % 2) * KCH * 128 + k0 * 128 + (k1 - k0) * 128]
                        .rearrange("p (k m) -> p k m", k=k1 - k0)[:, :, 0:int(nW[c_])],
                        w1_v[:, k0:k1, int(rc0[c_]):int(rc0[c_] + nW[c_])]
                        ).then_inc(dw, 16)
            dx = dxa if t_ % 2 == 0 else dxb
            if t_ >= 2:
                s.wait_ge(pk, t_ - 1)
            for i in range(NX):
                k0, k1 = (i * KCH) // NX, ((i + 1) * KCH) // NX
                s.dma_start(
                    xtb[:, (t_ % 2) * KCH * 128 + k0 * 128:
                        (t_ % 2) * KCH * 128 + k0 * 128 + (k1 - k0) * 128]
                    .rearrange("p (k m) -> p k m", k=k1 - k0)[:, :, 0:ml_],
                    xt_v[:, k0:k1, coloff[c_] + m0_:coloff[c_] + m0_ + ml_]
                    ).then_inc(dx, 16)
        # halves rows for the tail chunk's dense tile (buffer 1), after
        # q2dot(k=5) has finished reading that buffer
        s.wait_ge(psc, 71)
        s.dma_start(dense[1][104:109, 0:Bp], hv_ext[:, :]).then_inc(hv, 16)
        # hash-chain reshape dmas via DRAM bounce (partition reshapes)
        s.wait_ge(vq, len(ccs) + (1 if 128 * W2D > Bp else 0))
        s.dma_start(hscr_ext[:].rearrange("(o n) -> o n", o=1),
                    q2row[:, :]).then_inc(hd, 16)
        s.wait_ge(hd, 16)
        s.dma_start(hs[:, 0:W2D],
                    hscr_ext[:].rearrange("(p w) -> p w", p=128)).then_inc(hd, 16)
        s.wait_ge(hz, plan['HZF'])
        s.dma_start(pscr_ext[:].rearrange("(p w) -> p w", p=128),
                    hs[:, 11 * W2D:12 * W2D]).then_inc(hd, 16)
        s.wait_ge(hd, 48)
        s.dma_start(pmrow[:, :],
                    pscr_ext[:].rearrange("(o n) -> o n", o=1)).then_inc(hd, 16)
        # layer-2 weights, streamed per bucket into 2 rotating slots
        for c1 in range(10):
            if c1 >= 2:
                s.wait_ge(pe2, 16 * (c1 - 1))
            s.dma_start(w2sb[0:int(nW[c1]), (c1 % 2) * 1024:(c1 % 2) * 1024 + 1024],
                        w2_ext[int(rc0[c1]):int(rc0[c1] + nW[c1]), :]).then_inc(dw2, 16)

    # ---------------- PE engine ----------------
    @block.tensor
    def _(t):
        # layer 1: tile-outer chains (one open psum group per bank)
        first_tile = {}
        for t_, (c_, _, _) in enumerate(tiles):
            first_tile.setdefault(c_, t_)
        for ti, (c, m0, mlen) in enumerate(tiles):
            if first_tile[c] == ti:
                dw = dwa if c % 2 == 0 else dwb
                t.wait_ge(dw, (c // 2 + 1) * NWS * 16)
            dx = dxa if ti % 2 == 0 else dxb
            t.wait_ge(dx, (ti // 2 + 1) * NX * 16)
            if ti >= 2:
                t.wait_ge(ar, ti - 1)
            for k in range(KCH):
                mm = t.matmul(
                    banks[ti % 2][0:mlen, 0:int(nW[c])],
                    xtb[:, (ti % 2) * KCH * 128 + k * 128:
                        (ti % 2) * KCH * 128 + k * 128 + mlen],
                    wcb[:, (c % 2) * KCH * 128 + k * 128:
                        (c % 2) * KCH * 128 + k * 128 + int(nW[c])],
                    start=(k == 0), stop=(k == KCH - 1))
                if k == KCH - 1:
                    mm.then_inc(pk, 1)
        # transposes of relu'd blocks -> compact h1pT
        t.wait_ge(dsem, D0)
        for ti, (c, m0, mlen) in enumerate(tiles):
            t.wait_ge(ar, ti + 1)
            if ti >= 2:
                t.wait_ge(vc, ti - 1)  # bank (5 + ti%2) free when copy ti-2 done
            t.transpose(banks[5 + ti % 2][0:int(nW[c]), 0:mlen],
                        yblk[0:mlen, ybo[ti]:ybo[ti] + int(nW[c])],
                        idsb[0:mlen, 0:mlen]).then_inc(pt, 1)
        # scatter into dense h1^T chunk tiles (k-major, 2 rotating buffers)
        # interleaved with the q2 dot accumulation (a2-stationary, chunked
        # exactly like the reference's matvec)
        t.wait_ge(vc, len(tiles))
        si = 0
        for k in range(8):
            mrows = 128 if k < 7 else 104
            for c in range(10):
                if si >= 2:
                    t.wait_ge(vd, si - 1)
                t.matmul(banks[6 + si % 2][0:mrows, 0:int(mstar[c])],
                         smsb[0:int(nW[c]), c * H + k * 128:c * H + k * 128 + mrows],
                         hcomp[0:int(nW[c]), coloff[c]:coloff[c] + int(mstar[c])],
                         start=True, stop=True).then_inc(psc, 1)
                si += 1
            t.wait_ge(vd, 10 * (k + 1))
            if k == 7:
                t.wait_ge(hv, 16)
            ck = a2len[k]
            for cc, (co, cl) in enumerate(ccs):
                mm = t.matmul(banks[cc][0:1, 0:cl],
                              a2sb[0:ck, k:k + 1],
                              dense[k % 2][0:ck, co:co + cl],
                              start=(k == 0), stop=(k == 7))
                if k == 7:
                    mm.then_inc(pq, 1)
        # broadcast pm row to 128 partitions
        t.wait_ge(hd, 64)
        for cc, (co, cl) in enumerate(ccs):
            t.matmul(banks[5][0:128, 0:cl], onsb[:, :], pmrow[0:1, co:co + cl],
                     start=True, stop=True).then_inc(pb, 1)
            t.wait_ge(vb, cc + 1)  # single bank
        # layer 2 (masked) - banks 0/1 for even c1, 2/3 for odd
        t.wait_ge(vb, len(ccs))
        m16 = 0

        def emit_l3(j):
            bs = 0 if j % 2 == 0 else 4
            t.wait_ge(am, 16 * (j + 1))
            if j >= 1:
                t.wait_ge(vo, j)
            for mt in range(8):
                klen = min(128, H - mt * 128)
                mm = t.matmul(banks[bs + 3][0:O, 0:int(mstar[j])],
                              wosb[0:klen, mt * O:(mt + 1) * O],
                              h2t[0:klen, (j % 2) * 8 * 256 + mt * 256:
                                  (j % 2) * 8 * 256 + mt * 256 + int(mstar[j])],
                              start=(mt == 0), stop=(mt == 7))
                if mt == 7:
                    mm.then_inc(pe3, 1)

        for c1 in range(10):
            bset = 0 if c1 % 2 == 0 else 4
            t.wait_ge(dw2, min(160, (c1 + 2) * 16))
            if c1 >= 2:
                t.wait_ge(vm, 16 * (c1 - 1))
                t.wait_ge(vo, c1 - 1)
            for mt in range(8):
                for hf in range(2):
                    h0, hl = halves2[c1][hf]
                    if mt >= 4 and hf == 0:
                        t.wait_ge(vm, c1 * 16 + (mt - 4) * 2 + 2)
                    slot = ((mt // 4) * 2 + hf) * 128
                    t.matmul(banks[bset + mt % 4][0:128, slot:slot + hl],
                             w2sb[0:int(nW[c1]), (c1 % 2) * 1024 + mt * 128:(c1 % 2) * 1024 + (mt + 1) * 128],
                             hcomp[0:int(nW[c1]), coloff[c1] + h0:coloff[c1] + h0 + hl],
                             start=True, stop=True).then_inc(pe2, 1)
                    m16 += 1
            if c1 >= 1:
                emit_l3(c1 - 1)
        emit_l3(9)

    # ---------------- ACT engine ----------------
    @block.scalar
    def _(s):
        for ti, (c, m0, mlen) in enumerate(tiles):
            s.wait_ge(pk, ti + 1)
            s.activation(yblk[0:mlen, ybo[ti]:ybo[ti] + int(nW[c])],
                         banks[ti % 2][0:mlen, 0:int(nW[c])],
                         AF.Relu).then_inc(ar, 1)
        # hash chain: u = t * 10  (= t * RECIP(0.1), bit-identical)
        s.wait_ge(hz, 1)
        s.activation(hs[:, 2 * W2D:3 * W2D], hs[:, 1 * W2D:2 * W2D],
                     AF.Copy, scale=10.0).then_inc(ah, 1)
        # layer-2 relu
        m16 = 0
        for c1 in range(10):
            for mt in range(8):
                for hf in range(2):
                    h0, hl = halves2[c1][hf]
                    s.wait_ge(vm, m16 + 1)
                    s.activation(h2t[0:128, (c1 % 2) * 8 * 256 + mt * 256 + h0:
                                     (c1 % 2) * 8 * 256 + mt * 256 + h0 + hl],
                                 h2s[0:128, (m16 % 2) * 256:(m16 % 2) * 256 + hl],
                                 AF.Relu).then_inc(am, 1)
                    m16 += 1

    # ---------------- DVE engine ----------------
    @block.vector
    def _(v):
        # compact copies after transposes
        for ti, (c, m0, mlen) in enumerate(tiles):
            v.wait_ge(pt, ti + 1)
            v.tensor_copy(hcomp[0:int(nW[c]), coloff[c] + m0:coloff[c] + m0 + mlen],
                          banks[5 + ti % 2][0:int(nW[c]), 0:mlen]).then_inc(vc, 1)
        # dense copies after scatters (k-major)
        si = 0
        for k in range(8):
            mrows = 128 if k < 7 else 104
            for c in range(10):
                v.wait_ge(psc, si + 1)
                v.tensor_copy(dense[k % 2][0:mrows, coloff[c]:coloff[c] + int(mstar[c])],
                              banks[6 + si % 2][0:mrows, 0:int(mstar[c])]
                              ).then_inc(vd, 1)
                si += 1
        # q2 row assembly (+ pad-tail fill so the bounce DMA reads no
        # uninitialized bytes; pad lanes are discarded downstream)
        for cc, (co, cl) in enumerate(ccs):
            v.wait_ge(pq, cc + 1)
            v.tensor_copy(q2row[0:1, co:co + cl], banks[cc][0:1, 0:cl]).then_inc(vq, 1)
        if 128 * W2D > Bp:
            v.wait_ge(vq, len(ccs))
            v.tensor_copy(q2row[0:1, Bp:128 * W2D],
                          q2row[0:1, 0:128 * W2D - Bp]).then_inc(vq, 1)
        # hash chain on [128, W2D] layout (rows of hs: 0=q2,1=t,2=u,3=fi,4=g,
        # 5=f,6=fb,7=qa,8=qf,9=g2,10=qfl,11=pm; hsi: 0=i32,1=qi)
        hzc = 0

        def row(i):
            return hs[:, i * W2D:(i + 1) * W2D]

        v.wait_ge(hd, 32)
        v.tensor_scalar_add(row(1), row(0), bh2).then_inc(hz, 1); hzc += 1
        v.wait_ge(ah, 1)   # u on ACT
        v.tensor_copy(hsi[:, 0:W2D], row(2)).then_inc(hz, 1); hzc += 1
        v.wait_ge(hz, hzc)
        v.tensor_copy(row(3), hsi[:, 0:W2D]).then_inc(hz, 1); hzc += 1
        v.wait_ge(hz, hzc)
        v.tensor_tensor(row(4), row(3), row(2), op=ALU.is_gt).then_inc(hz, 1); hzc += 1
        v.wait_ge(hz, hzc)
        v.tensor_tensor(row(5), row(3), row(4), op=ALU.subtract).then_inc(hz, 1); hzc += 1
        v.wait_ge(hz, hzc)
        v.tensor_scalar_add(row(6), row(5), 0.5).then_inc(hz, 1); hzc += 1
        v.wait_ge(hz, hzc)
        v.tensor_scalar_mul(row(7), row(6), 0.1).then_inc(hz, 1); hzc += 1
        v.wait_ge(hz, hzc)
        v.tensor_copy(hsi[:, W2D:2 * W2D], row(7)).then_inc(hz, 1); hzc += 1
        v.wait_ge(hz, hzc)
        v.tensor_copy(row(8), hsi[:, W2D:2 * W2D]).then_inc(hz, 1); hzc += 1
        v.wait_ge(hz, hzc)
        v.tensor_tensor(row(9), row(8), row(7), op=ALU.is_gt).then_inc(hz, 1); hzc += 1
        v.wait_ge(hz, hzc)
        v.tensor_tensor(row(10), row(8), row(9), op=ALU.subtract).then_inc(hz, 1); hzc += 1
        v.wait_ge(hz, hzc)
        v.tensor_scalar(row(3), row(10), 10.0, None, op0=ALU.mult).then_inc(hz, 1); hzc += 1
        v.wait_ge(hz, hzc)
        v.tensor_tensor(row(11), row(5), row(3), op=ALU.subtract).then_inc(hz, 1); hzc += 1
        assert hzc == plan['HZF']
        # pm broadcast copies
        for cc, (co, cl) in enumerate(ccs):
            v.wait_ge(pb, cc + 1)
            v.tensor_copy(pmb[0:128, co:co + cl], banks[5][0:128, 0:cl]).then_inc(vb, 1)
        # layer-2 masking
        v.wait_ge(vb, len(ccs))
        m16 = 0
        hz2c = 0
        def out_copy(j):
            v.wait_ge(pe3, j + 1)
            v.tensor_copy(outsb[0:O, coloff[j]:coloff[j] + int(mstar[j])],
                          banks[(0 if j % 2 == 0 else 4) + 3][0:O, 0:int(mstar[j])]
                          ).then_inc(vo, 1)

        for c1 in range(10):
            bset = 0 if c1 % 2 == 0 else 4
            for mt in range(8):
                for hf in range(2):
                    h0, hl = halves2[c1][hf]
                    v.wait_ge(pe2, m16 + 1)
                    if m16 >= 2:
                        v.wait_ge(am, m16 - 1)
                    slot = ((mt // 4) * 2 + hf) * 128
                    v.tensor_scalar(eqs[0:128, (m16 % 2) * 256:(m16 % 2) * 256 + hl],
                                    pmb[0:128, coloff[c1] + h0:coloff[c1] + h0 + hl],
                                    rbsb[:, mt:mt + 1], None,
                                    op0=ALU.is_equal).then_inc(hz2, 1)
                    hz2c += 1
                    v.wait_ge(hz2, hz2c)
                    v.tensor_tensor(h2s[0:128, (m16 % 2) * 256:(m16 % 2) * 256 + hl],
                                    banks[bset + mt % 4][0:128, slot:slot + hl],
                                    eqs[0:128, (m16 % 2) * 256:(m16 % 2) * 256 + hl],
                                    op=ALU.mult).then_inc(vm, 1)
                    m16 += 1
            # out copy, shifted one bucket behind the pipelined layer-3
            if c1 >= 1:
                out_copy(c1 - 1)
        out_copy(9)

    # ---------------- gpsimd: output DMA ----------------
    @block.gpsimd
    def _(g):
        g.wait_ge(vo, 10)
        g.dma_start(out_ext[:, :], outsb[:, :]).then_inc(dout, 16)

    ctx.close()
    return nc


def kernel(x, W1, b1, W2, b2, Wout, bout, a1, bh1, a2, bh2):
    import sys
    if '/opt/trn_rl_repo' not in sys.path:
        sys.path.insert(0, '/opt/trn_rl_repo')
    from concourse.bass_utils import run_bass_kernel_spmd

    x = np.ascontiguousarray(x, np.float32)
    W1 = np.ascontiguousarray(W1, np.float32)
    W2 = np.ascontiguousarray(W2, np.float32)
    Wout = np.ascontiguousarray(Wout, np.float32)
    a1 = np.asarray(a1, np.float32)
    a2 = np.asarray(a2, np.float32)
    assert np.all(np.asarray(b1) == 0) and np.all(np.asarray(b2) == 0), \
        "kernel specialized for zero hidden biases"

    row_h1, row_h2, q_h1 = _host_hashes(x, W1, W2, a1, bh1, a2, bh2)

    perm1 = np.argsort(row_h1, kind='stable')
    perm2 = np.argsort(row_h2, kind='stable')
    nW = np.bincount(row_h1, minlength=10)
    rc0 = np.concatenate([[0], np.cumsum(nW)[:-1]])
    assert nW.max() <= 128 and nW.min() >= 1

    # deal samples: bucket-grouped, round-robin across cores, padded
    order = np.argsort(q_h1, kind='stable')
    qn = np.bincount(q_h1, minlength=10)
    qc0 = np.concatenate([[0], np.cumsum(qn)[:-1]])
    mstar = np.array([(qn[c] + NCORES - 1) // NCORES for c in range(10)], np.int64)
    assert mstar.max() <= 256
    coloff = np.concatenate([[0], np.cumsum(mstar)[:-1]]).astype(np.int64)
    Bp = int(mstar.sum())
    cols = np.full((NCORES, Bp), -1, np.int64)
    for c in range(10):
        idxs = order[qc0[c]:qc0[c] + qn[c]]
        for j in range(NCORES):
            part = idxs[j::NCORES]
            cols[j, coloff[c]:coloff[c] + len(part)] = part

    # layer-1 sample tiles (<=128) and psum packing over banks 0..4
    tiles = []
    for c in range(10):
        m = int(mstar[c])
        nt = (m + 127) // 128
        step = (m + nt - 1) // nt
        o = 0
        while o < m:
            tiles.append((c, o, min(step, m - o)))
            o += step
    l1bank = []
    bank_used = [0] * 5
    for (c, m0, mlen) in tiles:
        bsel = min(range(5), key=lambda b: bank_used[b])
        assert bank_used[bsel] + int(nW[c]) <= 512
        l1bank.append((bsel, bank_used[bsel]))
        bank_used[bsel] += int(nW[c])

    ncc = (Bp + 511) // 512
    assert ncc <= 5
    ccs = []
    for i in range(ncc):
        o = (i * Bp) // ncc
        e = ((i + 1) * Bp) // ncc
        ccs.append((o, e - o))
    W2D = (Bp + 127) // 128

    plan = dict(Bp=Bp, nW=nW, rc0=rc0, coloff=coloff, mstar=mstar, tiles=tiles,
                l1bank=l1bank, ccs=ccs, bh2=float(np.asarray(bh2, np.float32)),
                W2D=W2D, HZF=13)

    # shared host arrays
    W1pT = np.ascontiguousarray(W1[perm1].T)
    W2ppT = np.zeros((H, 1024), np.float32)
    W2ppT[:, :H] = W2.T[np.ix_(perm1, perm2)]
    WoT = np.ascontiguousarray(Wout[:, perm2].T)
    smats = []
    for c in range(10):
        Sc = np.zeros((int(nW[c]), H), np.float32)
        rows = perm1[rc0[c]:rc0[c] + nW[c]]
        Sc[np.arange(int(nW[c])), rows] = 1.0
        smats.append(Sc)
    a2c = np.ascontiguousarray(a2.reshape(-1, 1))
    rowb = np.full((128, 8), -1.0, np.float32)
    rbp = row_h2[perm2].astype(np.float32)
    for t in range(8):
        klen = min(128, H - t * 128)
        rowb[0:klen, t] = rbp[t * 128:t * 128 + klen]
    halves = np.full((5, Bp), 0.5, np.float32)
    ones = np.ones((1, 128), np.float32)
    ident = np.eye(128, dtype=np.float32)

    xTx = np.concatenate([x, np.zeros((1, D), np.float32)], axis=0)  # pad row
    in_maps = []
    for j in range(NCORES):
        idx = np.where(cols[j] >= 0, cols[j], B)
        xtj = np.ascontiguousarray(xTx[idx].T)  # [D, Bp]
        m = {"xt": xtj, "w1t": W1pT, "w2t": W2ppT, "wot": WoT,
             "a2c": a2c, "rowb": rowb, "halves": halves, "ones": ones,
             "ident": ident}
        for c in range(10):
            m[f"smat{c}"] = smats[c]
        in_maps.append(m)

    nc = _build_graph(plan)
    import os, time as _time
    trace = bool(os.environ.get("ALSH_TRACE"))
    if trace:
        try:
            res = run_bass_kernel_spmd(nc, in_maps, core_ids=list(range(NCORES)),
                                       trace=True)
            if res.exec_time_ns is not None:
                print(f"HW exec time: {res.exec_time_ns} ns", flush=True)
        except ModuleNotFoundError:
            # no NTFF hook in this environment: fall back to wall-clock of a
            # warm re-execution (upper bound: includes host<->device transfer)
            res = run_bass_kernel_spmd(nc, in_maps, core_ids=list(range(NCORES)))
            t0 = _time.time()
            res = run_bass_kernel_spmd(nc, in_maps, core_ids=list(range(NCORES)))
            wall = (_time.time() - t0) * 1e9
            print(f"HW exec time: {wall:.0f} ns (wall-clock upper bound, "
                  f"incl. transfers; no NTFF hook available)", flush=True)
    else:
        res = run_bass_kernel_spmd(nc, in_maps, core_ids=list(range(NCORES)))

    out = np.zeros((B, O), np.float32)
    for j in range(NCORES):
        oj = res.results[j]["out"]  # [O, Bp]
        sel = cols[j] >= 0
        out[cols[j][sel]] = oj[:, sel].T
    return out + np.asarray(bout, np.float32)


# revision 20
# speedup vs baseline: 1.1588x; 1.1588x over previous
"""ALSH net forward on 8 Trainium2 NeuronCores (bass, SPMD data-parallel).

Strategy
--------
Data-parallel over the batch.  Samples are grouped by their layer-1 hash
bucket on the host (hashes computed with the exact same jax ops as the
reference so the bucket assignments are bit-identical), then dealt
round-robin to the 8 cores so every core sees the same per-bucket counts
(padded) and a single SPMD graph serves all cores.

Per core the kernel exploits the ALSH block-sparsity: a sample only needs
the weight rows in its own bucket, so layer 1 runs ~10x fewer FLOPs than
the dense matmul.  Numerical reproducibility (hash buckets flip on 1-ulp
differences) is handled by replicating the neuron-XLA fp32 matmul
structure exactly: x-tiles stationary, K chunked by 128 in ascending
order, PSUM-accumulated; the layer-2 query hash is computed from a
densely scattered h1^T (built with bit-preserving 0/1 scatter matmuls)
with the same chunk structure the reference's matvec uses.
"""

import numpy as np

R = 0.1
TABLE = 10
M = 5
NCORES = 8
B, D, H, O = 16384, 3072, 1000, 10
KCH = D // 128  # 24 K chunks for layer 1


def _host_hashes(x, W1, W2, a1, bh1, a2, bh2):
    """Bit-exact replica of the reference hash computations (same jnp ops,
    same neuron backend) for the quantities that only depend on inputs."""
    import jax.numpy as jnp

    def _P(w):
        n = jnp.linalg.norm(w, axis=-1, keepdims=True)
        powers = jnp.concatenate([n ** (2 ** (i + 1)) for i in range(M)], axis=-1)
        return jnp.concatenate([w, powers], axis=-1)

    def _Q(xx):
        halves = jnp.full(xx.shape[:-1] + (M,), 0.5, xx.dtype)
        return jnp.concatenate([xx, halves], axis=-1)

    def _hash(v, a, b):
        h = jnp.floor((v @ a + b) / R)
        return h.astype(jnp.int32) % TABLE

    row_h1 = np.asarray(_hash(_P(jnp.asarray(W1)), jnp.asarray(a1), jnp.asarray(bh1)))
    row_h2 = np.asarray(_hash(_P(jnp.asarray(W2)), jnp.asarray(a2), jnp.asarray(bh2)))
    q_h1 = np.asarray(_hash(_Q(jnp.asarray(x)), jnp.asarray(a1), jnp.asarray(bh1)))
    return row_h1, row_h2, q_h1


def _build_graph(plan):
    import sys
    if '/opt/trn_rl_repo' not in sys.path:
        sys.path.insert(0, '/opt/trn_rl_repo')
    import concourse.bass as bass
    import concourse.mybir as mybir
    from contextlib import ExitStack

    F32 = mybir.dt.float32
    I32 = mybir.dt.int32
    AF = mybir.ActivationFunctionType
    ALU = mybir.AluOpType

    Bp = plan['Bp']
    nW = plan['nW']            # [10] rows per bucket (layer1, perm1 order)
    rc0 = plan['rc0']          # [10] start row of bucket in perm1 order
    coloff = plan['coloff']    # [10] per-core sample column offset per bucket
    mstar = plan['mstar']      # [10] per-core padded sample count per bucket
    tiles = plan['tiles']      # [(c, m0, mlen)] layer-1 sample tiles (20)
    l1bank = plan['l1bank']    # per tile: (bank, freeoff)
    ccs = plan['ccs']          # [(off, len)] q2dot column chunks over Bp
    bh2 = plan['bh2']
    W2D = plan['W2D']          # hash-chain 2d width = ceil(Bp/128)
    NX, NWS = 6, 3             # x / w1t dma splits per tile/bucket
    a2len = [128] * 7 + [109]

    nc = bass.Bass()
    xt_ext = nc.dram_tensor("xt", [D, Bp], F32, kind="ExternalInput")
    w1_ext = nc.dram_tensor("w1t", [D, H], F32, kind="ExternalInput")
    w2_ext = nc.dram_tensor("w2t", [H, 1024], F32, kind="ExternalInput")
    wo_ext = nc.dram_tensor("wot", [H, O], F32, kind="ExternalInput")
    sm_ext = [nc.dram_tensor(f"smat{c}", [int(nW[c]), H], F32, kind="ExternalInput")
              for c in range(10)]
    a2_ext = nc.dram_tensor("a2c", [1005, 1], F32, kind="ExternalInput")
    rb_ext = nc.dram_tensor("rowb", [128, 8], F32, kind="ExternalInput")
    hv_ext = nc.dram_tensor("halves", [5, Bp], F32, kind="ExternalInput")
    on_ext = nc.dram_tensor("ones", [1, 128], F32, kind="ExternalInput")
    id_ext = nc.dram_tensor("ident", [128, 128], F32, kind="ExternalInput")
    out_ext = nc.dram_tensor("out", [O, Bp], F32, kind="ExternalOutput")
    hscr_ext = nc.dram_tensor("hscr", [128 * W2D], F32)
    pscr_ext = nc.dram_tensor("pscr", [128 * W2D], F32)

    ctx = ExitStack()
    ec = ctx.enter_context
    xtb = ec(nc.sbuf_tensor("xtb", [128, 2 * KCH * 128], F32))
    wcb = ec(nc.sbuf_tensor("wcb", [128, 2 * KCH * 128], F32))
    idsb = ec(nc.sbuf_tensor("idsb", [128, 128], F32))
    onsb = ec(nc.sbuf_tensor("onsb", [1, 128], F32))
    rbsb = ec(nc.sbuf_tensor("rbsb", [128, 8], F32))
    wosb = ec(nc.sbuf_tensor("wosb", [128, 8 * O], F32))
    w2sb = ec(nc.sbuf_tensor("w2sb", [128, 3 * 1024], F32))
    smsb = ec(nc.sbuf_tensor("smsb", [128, 10 * H], F32))
    a2sb = ec(nc.sbuf_tensor("a2sb", [128, 8], F32))
    yblk = ec(nc.sbuf_tensor("yblk", [128, sum(int(nW[c]) for c, _, _ in tiles)], F32))
    hcomp = ec(nc.sbuf_tensor("hcomp", [128, Bp], F32))
    dense = [ec(nc.sbuf_tensor(f"dense{k}", [128, Bp], F32)) for k in range(2)]
    q2row = ec(nc.sbuf_tensor("q2row", [1, 128 * W2D], F32))
    pmrow = ec(nc.sbuf_tensor("pmrow", [1, 128 * W2D], F32))
    hs = ec(nc.sbuf_tensor("hs", [128, 12 * W2D], F32))
    hsi = ec(nc.sbuf_tensor("hsi", [128, 2 * W2D], I32))
    pmb = ec(nc.sbuf_tensor("pmb", [128, Bp], F32))
    eqs = ec(nc.sbuf_tensor("eqs", [128, 2 * 256], F32))
    h2s = ec(nc.sbuf_tensor("h2s", [128, 2 * 256], F32))
    h2t = ec(nc.sbuf_tensor("h2t", [128, 2 * 8 * 256], F32))
    outsb = ec(nc.sbuf_tensor("outsb", [10, Bp], F32))
    banks = [ec(nc.psum_tensor(f"bank{i}", [128, 512], F32)) for i in range(8)]

    dsem = ec(nc.semaphore("dsem"))
    hd = ec(nc.semaphore("hd"))
    pk = ec(nc.semaphore("pk"))
    ar = ec(nc.semaphore("ar"))
    pt = ec(nc.semaphore("pt"))
    vc = ec(nc.semaphore("vc"))
    psc = ec(nc.semaphore("psc"))
    vd = ec(nc.semaphore("vd"))
    pq = ec(nc.semaphore("pq"))
    vq = ec(nc.semaphore("vq"))
    hz = ec(nc.semaphore("hz"))
    ah = ec(nc.semaphore("ah"))
    pb = ec(nc.semaphore("pb"))
    vb = ec(nc.semaphore("vb"))
    pe2 = ec(nc.semaphore("pe2"))
    hz2 = ec(nc.semaphore("hz2"))
    vm = ec(nc.semaphore("vm"))
    am = ec(nc.semaphore("am"))
    pe3 = ec(nc.semaphore("pe3"))
    vo = ec(nc.semaphore("vo"))
    hv = ec(nc.semaphore("hv"))
    dxa = ec(nc.semaphore("dxa"))
    dxb = ec(nc.semaphore("dxb"))
    dwa = ec(nc.semaphore("dwa"))
    dwb = ec(nc.semaphore("dwb"))
    dw2 = ec(nc.semaphore("dw2"))
    dout = ec(nc.semaphore("dout"))
    block = ec(nc.Block())

    # yblk free offsets per tile
    ybo = []
    o_ = 0
    for (c, m0, mlen) in tiles:
        ybo.append(o_)
        o_ += int(nW[c])

    # layer-2 halves of each bucket's sample columns
    halves2 = []
    for c in range(10):
        m = int(mstar[c])
        h0 = (m + 1) // 2
        halves2.append([(0, h0), (h0, m - h0)])

    D0 = 29 * 16  # phase-0 dma completions

    # ---------------- sync engine: all input DMAs ----------------
    @block.sync
    def _(s):
        s.dma_start(idsb[:, :], id_ext[:, :]).then_inc(dsem, 16)
        s.dma_start(onsb[:, :], on_ext[:, :]).then_inc(dsem, 16)
        s.dma_start(rbsb[:, :], rb_ext[:, :]).then_inc(dsem, 16)
        for t in range(8):
            klen = min(128, H - t * 128)
            s.dma_start(wosb[0:klen, t * O:(t + 1) * O],
                        wo_ext[t * 128:t * 128 + klen, :]).then_inc(dsem, 16)
        for c in range(10):
            s.dma_start(smsb[0:int(nW[c]), c * H:(c + 1) * H],
                        sm_ext[c][:, :]).then_inc(dsem, 16)
        for k in range(8):
            ck = a2len[k]
            s.dma_start(a2sb[0:ck, k:k + 1],
                        a2_ext[k * 128:k * 128 + ck, :]).then_inc(dsem, 16)
        # streamed layer-1 tiles: x columns per tile, W columns per bucket
        xt_v = xt_ext.rearrange("(k p) m -> p k m", p=128)
        w1_v = w1_ext.rearrange("(k p) m -> p k m", p=128)
        first_tile = {}
        last_tile = {}
        for t_, (c_, _, _) in enumerate(tiles):
            first_tile.setdefault(c_, t_)
            last_tile[c_] = t_
        for t_, (c_, m0_, ml_) in enumerate(tiles):
            if first_tile[c_] == t_:
                dw = dwa if c_ % 2 == 0 else dwb
                if c_ >= 2:
                    s.wait_ge(pk, last_tile[c_ - 2] + 1)
                for i in range(NWS):
                    k0, k1 = (i * KCH) // NWS, ((i + 1) * KCH) // NWS
                    s.dma_start(
                        wcb[:, (c_ % 2) * KCH * 128 + k0 * 128:
                            (c_ % 2) * KCH * 128 + k0 * 128 + (k1 - k0) * 128]
                        .rearrange("p (k m) -> p k m", k=k1 - k0)[:, :, 0:int(nW[c_])],
                        w1_v[:, k0:k1, int(rc0[c_]):int(rc0[c_] + nW[c_])]
                        ).then_inc(dw, 16)
            dx = dxa if t_ % 2 == 0 else dxb
            if t_ >= 2:
                s.wait_ge(pk, t_ - 1)
            for i in range(NX):
                k0, k1 = (i * KCH) // NX, ((i + 1) * KCH) // NX
                s.dma_start(
                    xtb[:, (t_ % 2) * KCH * 128 + k0 * 128:
                        (t_ % 2) * KCH * 128 + k0 * 128 + (k1 - k0) * 128]
                    .rearrange("p (k m) -> p k m", k=k1 - k0)[:, :, 0:ml_],
                    xt_v[:, k0:k1, coloff[c_] + m0_:coloff[c_] + m0_ + ml_]
                    ).then_inc(dx, 16)
        # halves rows for the tail chunk's dense tile (buffer 1), after
        # q2dot(k=5) has finished reading that buffer
        s.wait_ge(psc, 71)
        s.dma_start(dense[1][104:109, 0:Bp], hv_ext[:, :]).then_inc(hv, 16)
        # hash-chain reshape dmas via DRAM bounce (partition reshapes)
        s.wait_ge(vq, len(ccs) + (1 if 128 * W2D > Bp else 0))
        s.dma_start(hscr_ext[:].rearrange("(o n) -> o n", o=1),
                    q2row[:, :]).then_inc(hd, 16)
        s.wait_ge(hd, 16)
        s.dma_start(hs[:, 0:W2D],
                    hscr_ext[:].rearrange("(p w) -> p w", p=128)).then_inc(hd, 16)
        s.wait_ge(hz, plan['HZF'])
        s.dma_start(pscr_ext[:].rearrange("(p w) -> p w", p=128),
                    hs[:, 11 * W2D:12 * W2D]).then_inc(hd, 16)
        s.wait_ge(hd, 48)
        s.dma_start(pmrow[:, :],
                    pscr_ext[:].rearrange("(o n) -> o n", o=1)).then_inc(hd, 16)
        # layer-2 weights, streamed per bucket into 3 rotating slots
        for c1 in range(10):
            if c1 >= 3:
                s.wait_ge(pe2, 16 * (c1 - 2))
            for sp in range(2):
                s.dma_start(w2sb[0:int(nW[c1]), (c1 % 3) * 1024 + sp * 512:
                                 (c1 % 3) * 1024 + (sp + 1) * 512],
                            w2_ext[int(rc0[c1]):int(rc0[c1] + nW[c1]),
                                   sp * 512:(sp + 1) * 512]).then_inc(dw2, 16)

    # ---------------- PE engine ----------------
    @block.tensor
    def _(t):
        # layer 1: tile-outer chains (one open psum group per bank)
        first_tile = {}
        for t_, (c_, _, _) in enumerate(tiles):
            first_tile.setdefault(c_, t_)
        for ti, (c, m0, mlen) in enumerate(tiles):
            if first_tile[c] == ti:
                dw = dwa if c % 2 == 0 else dwb
                t.wait_ge(dw, (c // 2 + 1) * NWS * 16)
            dx = dxa if ti % 2 == 0 else dxb
            t.wait_ge(dx, (ti // 2 + 1) * NX * 16)
            if ti >= 2:
                t.wait_ge(ar, ti - 1)
            for k in range(KCH):
                mm = t.matmul(
                    banks[ti % 2][0:mlen, 0:int(nW[c])],
                    xtb[:, (ti % 2) * KCH * 128 + k * 128:
                        (ti % 2) * KCH * 128 + k * 128 + mlen],
                    wcb[:, (c % 2) * KCH * 128 + k * 128:
                        (c % 2) * KCH * 128 + k * 128 + int(nW[c])],
                    start=(k == 0), stop=(k == KCH - 1))
                if k == KCH - 1:
                    mm.then_inc(pk, 1)
        # transposes of relu'd blocks -> compact h1pT
        t.wait_ge(dsem, D0)
        for ti, (c, m0, mlen) in enumerate(tiles):
            t.wait_ge(ar, ti + 1)
            if ti >= 2:
                t.wait_ge(vc, ti - 1)  # bank (5 + ti%2) free when copy ti-2 done
            t.transpose(banks[5 + ti % 2][0:int(nW[c]), 0:mlen],
                        yblk[0:mlen, ybo[ti]:ybo[ti] + int(nW[c])],
                        idsb[0:mlen, 0:mlen]).then_inc(pt, 1)
        # scatter into dense h1^T chunk tiles (k-major, 2 rotating buffers)
        # interleaved with the q2 dot accumulation (a2-stationary, chunked
        # exactly like the reference's matvec)
        t.wait_ge(vc, len(tiles))
        si = 0
        for k in range(8):
            mrows = 128 if k < 7 else 104
            for c in range(10):
                if si >= 2:
                    t.wait_ge(vd, si - 1)
                t.matmul(banks[6 + si % 2][0:mrows, 0:int(mstar[c])],
                         smsb[0:int(nW[c]), c * H + k * 128:c * H + k * 128 + mrows],
                         hcomp[0:int(nW[c]), coloff[c]:coloff[c] + int(mstar[c])],
                         start=True, stop=True).then_inc(psc, 1)
                si += 1
            t.wait_ge(vd, 10 * (k + 1))
            if k == 7:
                t.wait_ge(hv, 16)
            ck = a2len[k]
            for cc, (co, cl) in enumerate(ccs):
                mm = t.matmul(banks[cc][0:1, 0:cl],
                              a2sb[0:ck, k:k + 1],
                              dense[k % 2][0:ck, co:co + cl],
                              start=(k == 0), stop=(k == 7))
                if k == 7:
                    mm.then_inc(pq, 1)
        # broadcast pm row to 128 partitions
        t.wait_ge(hd, 64)
        for cc, (co, cl) in enumerate(ccs):
            t.matmul(banks[5][0:128, 0:cl], onsb[:, :], pmrow[0:1, co:co + cl],
                     start=True, stop=True).then_inc(pb, 1)
            t.wait_ge(vb, cc + 1)  # single bank
        # layer 2 (masked) - banks 0/1 for even c1, 2/3 for odd
        t.wait_ge(vb, len(ccs))
        m16 = 0

        def emit_l3(j):
            bs = 0 if j % 2 == 0 else 4
            t.wait_ge(am, 16 * (j + 1))
            if j >= 1:
                t.wait_ge(vo, j)
            for mt in range(8):
                klen = min(128, H - mt * 128)
                mm = t.matmul(banks[bs + 3][0:O, 0:int(mstar[j])],
                              wosb[0:klen, mt * O:(mt + 1) * O],
                              h2t[0:klen, (j % 2) * 8 * 256 + mt * 256:
                                  (j % 2) * 8 * 256 + mt * 256 + int(mstar[j])],
                              start=(mt == 0), stop=(mt == 7))
                if mt == 7:
                    mm.then_inc(pe3, 1)

        for c1 in range(10):
            bset = 0 if c1 % 2 == 0 else 4
            t.wait_ge(dw2, min(320, (c1 + 3) * 32))
            if c1 >= 2:
                t.wait_ge(vm, 16 * (c1 - 1))
                t.wait_ge(vo, c1 - 1)
            for mt in range(8):
                for hf in range(2):
                    h0, hl = halves2[c1][hf]
                    if mt >= 4 and hf == 0:
                        t.wait_ge(vm, c1 * 16 + (mt - 4) * 2 + 2)
                    slot = ((mt // 4) * 2 + hf) * 128
                    t.matmul(banks[bset + mt % 4][0:128, slot:slot + hl],
                             w2sb[0:int(nW[c1]), (c1 % 3) * 1024 + mt * 128:(c1 % 3) * 1024 + (mt + 1) * 128],
                             hcomp[0:int(nW[c1]), coloff[c1] + h0:coloff[c1] + h0 + hl],
                             start=True, stop=True).then_inc(pe2, 1)
                    m16 += 1
            if c1 >= 1:
                emit_l3(c1 - 1)
        emit_l3(9)

    # ---------------- ACT engine ----------------
    @block.scalar
    def _(s):
        for ti, (c, m0, mlen) in enumerate(tiles):
            s.wait_ge(pk, ti + 1)
            s.activation(yblk[0:mlen, ybo[ti]:ybo[ti] + int(nW[c])],
                         banks[ti % 2][0:mlen, 0:int(nW[c])],
                         AF.Relu).then_inc(ar, 1)
        # hash chain: u = t * 10  (= t * RECIP(0.1), bit-identical)
        s.wait_ge(hz, 1)
        s.activation(hs[:, 2 * W2D:3 * W2D], hs[:, 1 * W2D:2 * W2D],
                     AF.Copy, scale=10.0).then_inc(ah, 1)
        # layer-2 relu
        m16 = 0
        for c1 in range(10):
            for mt in range(8):
                for hf in range(2):
                    h0, hl = halves2[c1][hf]
                    s.wait_ge(vm, m16 + 1)
                    s.activation(h2t[0:128, (c1 % 2) * 8 * 256 + mt * 256 + h0:
                                     (c1 % 2) * 8 * 256 + mt * 256 + h0 + hl],
                                 h2s[0:128, (m16 % 2) * 256:(m16 % 2) * 256 + hl],
                                 AF.Relu).then_inc(am, 1)
                    m16 += 1

    # ---------------- DVE engine ----------------
    @block.vector
    def _(v):
        # compact copies after transposes
        for ti, (c, m0, mlen) in enumerate(tiles):
            v.wait_ge(pt, ti + 1)
            v.tensor_copy(hcomp[0:int(nW[c]), coloff[c] + m0:coloff[c] + m0 + mlen],
                          banks[5 + ti % 2][0:int(nW[c]), 0:mlen]).then_inc(vc, 1)
        # dense copies after scatters (k-major)
        si = 0
        for k in range(8):
            mrows = 128 if k < 7 else 104
            for c in range(10):
                v.wait_ge(psc, si + 1)
                v.tensor_copy(dense[k % 2][0:mrows, coloff[c]:coloff[c] + int(mstar[c])],
                              banks[6 + si % 2][0:mrows, 0:int(mstar[c])]
                              ).then_inc(vd, 1)
                si += 1
        # q2 row assembly (+ pad-tail fill so the bounce DMA reads no
        # uninitialized bytes; pad lanes are discarded downstream)
        for cc, (co, cl) in enumerate(ccs):
            v.wait_ge(pq, cc + 1)
            v.tensor_copy(q2row[0:1, co:co + cl], banks[cc][0:1, 0:cl]).then_inc(vq, 1)
        if 128 * W2D > Bp:
            v.wait_ge(vq, len(ccs))
            v.tensor_copy(q2row[0:1, Bp:128 * W2D],
                          q2row[0:1, 0:128 * W2D - Bp]).then_inc(vq, 1)
        # hash chain on [128, W2D] layout (rows of hs: 0=q2,1=t,2=u,3=fi,4=g,
        # 5=f,6=fb,7=qa,8=qf,9=g2,10=qfl,11=pm; hsi: 0=i32,1=qi)
        hzc = 0

        def row(i):
            return hs[:, i * W2D:(i + 1) * W2D]

        v.wait_ge(hd, 32)
        v.tensor_scalar_add(row(1), row(0), bh2).then_inc(hz, 1); hzc += 1
        v.wait_ge(ah, 1)   # u on ACT
        v.tensor_copy(hsi[:, 0:W2D], row(2)).then_inc(hz, 1); hzc += 1
        v.wait_ge(hz, hzc)
        v.tensor_copy(row(3), hsi[:, 0:W2D]).then_inc(hz, 1); hzc += 1
        v.wait_ge(hz, hzc)
        v.tensor_tensor(row(4), row(3), row(2), op=ALU.is_gt).then_inc(hz, 1); hzc += 1
        v.wait_ge(hz, hzc)
        v.tensor_tensor(row(5), row(3), row(4), op=ALU.subtract).then_inc(hz, 1); hzc += 1
        v.wait_ge(hz, hzc)
        v.tensor_scalar_add(row(6), row(5), 0.5).then_inc(hz, 1); hzc += 1
        v.wait_ge(hz, hzc)
        v.tensor_scalar_mul(row(7), row(6), 0.1).then_inc(hz, 1); hzc += 1
        v.wait_ge(hz, hzc)
        v.tensor_copy(hsi[:, W2D:2 * W2D], row(7)).then_inc(hz, 1); hzc += 1
        v.wait_ge(hz, hzc)
        v.tensor_copy(row(8), hsi[:, W2D:2 * W2D]).then_inc(hz, 1); hzc += 1
        v.wait_ge(hz, hzc)
        v.tensor_tensor(row(9), row(8), row(7), op=ALU.is_gt).then_inc(hz, 1); hzc += 1
        v.wait_ge(hz, hzc)
        v.tensor_tensor(row(10), row(8), row(9), op=ALU.subtract).then_inc(hz, 1); hzc += 1
        v.wait_ge(hz, hzc)
        v.tensor_scalar(row(3), row(10), 10.0, None, op0=ALU.mult).then_inc(hz, 1); hzc += 1
        v.wait_ge(hz, hzc)
        v.tensor_tensor(row(11), row(5), row(3), op=ALU.subtract).then_inc(hz, 1); hzc += 1
        assert hzc == plan['HZF']
        # pm broadcast copies
        for cc, (co, cl) in enumerate(ccs):
            v.wait_ge(pb, cc + 1)
            v.tensor_copy(pmb[0:128, co:co + cl], banks[5][0:128, 0:cl]).then_inc(vb, 1)
        # layer-2 masking
        v.wait_ge(vb, len(ccs))
        m16 = 0
        hz2c = 0
        def out_copy(j):
            v.wait_ge(pe3, j + 1)
            v.tensor_copy(outsb[0:O, coloff[j]:coloff[j] + int(mstar[j])],
                          banks[(0 if j % 2 == 0 else 4) + 3][0:O, 0:int(mstar[j])]
                          ).then_inc(vo, 1)

        for c1 in range(10):
            bset = 0 if c1 % 2 == 0 else 4
            for mt in range(8):
                for hf in range(2):
                    h0, hl = halves2[c1][hf]
                    v.wait_ge(pe2, m16 + 1)
                    if m16 >= 2:
                        v.wait_ge(am, m16 - 1)
                    slot = ((mt // 4) * 2 + hf) * 128
                    v.tensor_scalar(eqs[0:128, (m16 % 2) * 256:(m16 % 2) * 256 + hl],
                                    pmb[0:128, coloff[c1] + h0:coloff[c1] + h0 + hl],
                                    rbsb[:, mt:mt + 1], None,
                                    op0=ALU.is_equal).then_inc(hz2, 1)
                    hz2c += 1
                    v.wait_ge(hz2, hz2c)
                    v.tensor_tensor(h2s[0:128, (m16 % 2) * 256:(m16 % 2) * 256 + hl],
                                    banks[bset + mt % 4][0:128, slot:slot + hl],
                                    eqs[0:128, (m16 % 2) * 256:(m16 % 2) * 256 + hl],
                                    op=ALU.mult).then_inc(vm, 1)
                    m16 += 1
            # out copy, shifted one bucket behind the pipelined layer-3
            if c1 >= 1:
                out_copy(c1 - 1)
        out_copy(9)

    # ---------------- gpsimd: output DMA ----------------
    @block.gpsimd
    def _(g):
        g.wait_ge(vo, 10)
        g.dma_start(out_ext[:, :], outsb[:, :]).then_inc(dout, 16)

    ctx.close()
    return nc


def kernel(x, W1, b1, W2, b2, Wout, bout, a1, bh1, a2, bh2):
    import sys
    if '/opt/trn_rl_repo' not in sys.path:
        sys.path.insert(0, '/opt/trn_rl_repo')
    from concourse.bass_utils import run_bass_kernel_spmd

    x = np.ascontiguousarray(x, np.float32)
    W1 = np.ascontiguousarray(W1, np.float32)
    W2 = np.ascontiguousarray(W2, np.float32)
    Wout = np.ascontiguousarray(Wout, np.float32)
    a1 = np.asarray(a1, np.float32)
    a2 = np.asarray(a2, np.float32)
    assert np.all(np.asarray(b1) == 0) and np.all(np.asarray(b2) == 0), \
        "kernel specialized for zero hidden biases"

    row_h1, row_h2, q_h1 = _host_hashes(x, W1, W2, a1, bh1, a2, bh2)

    perm1 = np.argsort(row_h1, kind='stable')
    perm2 = np.argsort(row_h2, kind='stable')
    nW = np.bincount(row_h1, minlength=10)
    rc0 = np.concatenate([[0], np.cumsum(nW)[:-1]])
    assert nW.max() <= 128 and nW.min() >= 1

    # deal samples: bucket-grouped, round-robin across cores, padded
    order = np.argsort(q_h1, kind='stable')
    qn = np.bincount(q_h1, minlength=10)
    qc0 = np.concatenate([[0], np.cumsum(qn)[:-1]])
    mstar = np.array([(qn[c] + NCORES - 1) // NCORES for c in range(10)], np.int64)
    assert mstar.max() <= 256
    coloff = np.concatenate([[0], np.cumsum(mstar)[:-1]]).astype(np.int64)
    Bp = int(mstar.sum())
    cols = np.full((NCORES, Bp), -1, np.int64)
    for c in range(10):
        idxs = order[qc0[c]:qc0[c] + qn[c]]
        for j in range(NCORES):
            part = idxs[j::NCORES]
            cols[j, coloff[c]:coloff[c] + len(part)] = part

    # layer-1 sample tiles (<=128) and psum packing over banks 0..4
    tiles = []
    for c in range(10):
        m = int(mstar[c])
        nt = (m + 127) // 128
        step = (m + nt - 1) // nt
        o = 0
        while o < m:
            tiles.append((c, o, min(step, m - o)))
            o += step
    l1bank = []
    bank_used = [0] * 5
    for (c, m0, mlen) in tiles:
        bsel = min(range(5), key=lambda b: bank_used[b])
        assert bank_used[bsel] + int(nW[c]) <= 512
        l1bank.append((bsel, bank_used[bsel]))
        bank_used[bsel] += int(nW[c])

    ncc = (Bp + 511) // 512
    assert ncc <= 5
    ccs = []
    for i in range(ncc):
        o = (i * Bp) // ncc
        e = ((i + 1) * Bp) // ncc
        ccs.append((o, e - o))
    W2D = (Bp + 127) // 128

    plan = dict(Bp=Bp, nW=nW, rc0=rc0, coloff=coloff, mstar=mstar, tiles=tiles,
                l1bank=l1bank, ccs=ccs, bh2=float(np.asarray(bh2, np.float32)),
                W2D=W2D, HZF=13)

    # shared host arrays
    W1pT = np.ascontiguousarray(W1[perm1].T)
    W2ppT = np.zeros((H, 1024), np.float32)
    W2ppT[:, :H] = W2.T[np.ix_(perm1, perm2)]
    WoT = np.ascontiguousarray(Wout[:, perm2].T)
    smats = []
    for c in range(10):
        Sc = np.zeros((int(nW[c]), H), np.float32)
        rows = perm1[rc0[c]:rc0[c] + nW[c]]
        Sc[np.arange(int(nW[c])), rows] = 1.0
        smats.append(Sc)
    a2c = np.ascontiguousarray(a2.reshape(-1, 1))
    rowb = np.full((128, 8), -1.0, np.float32)
    rbp = row_h2[perm2].astype(np.float32)
    for t in range(8):
        klen = min(128, H - t * 128)
        rowb[0:klen, t] = rbp[t * 128:t * 128 + klen]
    halves = np.full((5, Bp), 0.5, np.float32)
    ones = np.ones((1, 128), np.float32)
    ident = np.eye(128, dtype=np.float32)

    xTx = np.concatenate([x, np.zeros((1, D), np.float32)], axis=0)  # pad row
    in_maps = []
    for j in range(NCORES):
        idx = np.where(cols[j] >= 0, cols[j], B)
        xtj = np.ascontiguousarray(xTx[idx].T)  # [D, Bp]
        m = {"xt": xtj, "w1t": W1pT, "w2t": W2ppT, "wot": WoT,
             "a2c": a2c, "rowb": rowb, "halves": halves, "ones": ones,
             "ident": ident}
        for c in range(10):
            m[f"smat{c}"] = smats[c]
        in_maps.append(m)

    nc = _build_graph(plan)
    import os, time as _time
    trace = bool(os.environ.get("ALSH_TRACE"))
    if trace:
        try:
            res = run_bass_kernel_spmd(nc, in_maps, core_ids=list(range(NCORES)),
                                       trace=True)
            if res.exec_time_ns is not None:
                print(f"HW exec time: {res.exec_time_ns} ns", flush=True)
        except ModuleNotFoundError:
            # no NTFF hook in this environment: fall back to wall-clock of a
            # warm re-execution (upper bound: includes host<->device transfer)
            res = run_bass_kernel_spmd(nc, in_maps, core_ids=list(range(NCORES)))
            t0 = _time.time()
            res = run_bass_kernel_spmd(nc, in_maps, core_ids=list(range(NCORES)))
            wall = (_time.time() - t0) * 1e9
            print(f"HW exec time: {wall:.0f} ns (wall-clock upper bound, "
                  f"incl. transfers; no NTFF hook available)", flush=True)
    else:
        res = run_bass_kernel_spmd(nc, in_maps, core_ids=list(range(NCORES)))

    out = np.zeros((B, O), np.float32)
    for j in range(NCORES):
        oj = res.results[j]["out"]  # [O, Bp]
        sel = cols[j] >= 0
        out[cols[j][sel]] = oj[:, sel].T
    return out + np.asarray(bout, np.float32)
